# revision 8
# baseline (speedup 1.0000x reference)
"""Dense bilateral energy loss (DenseEnergyLoss) on 8 Trainium2 cores.

Math (per image n, after 2x downsample => oh=ow=64, P=4096):
  feat[p] = (x/40, y/40, r/15, g/15, b/15)          # 5 dims
  A[p,q]  = exp(-0.5*||feat_p - feat_q||^2)          # dense [P,P], SYMMETRIC
  loss    = -0.05 * sum_k t_k^T A u_k / (N*P)        # t = seg_m*gate, u = seg_m

Only the upper block-triangle of A at 128-row granularity is computed:
p-block pb needs columns from its diagonal onward.  Two cores per image
split the 32 p-blocks by parity (34816 A-columns = 4.46M exp elements
each); the diagonal chunk start is aligned to an EVEN sub-block so both
parities run one SPMD program, with the first two 128-col segments
reading per-core stationary DATA variants: S1 = [t/2;u/2] | zeros,
S2 = [t;u] | [t/2;u/2] (halved diagonal content makes V*u + W*t count
every unordered pair exactly once; below-diagonal reads zeros).

Per band-pair group g (bands 2g,2g+1; one PSUM accumulator bank with the
two bands at partition offsets 0/64), each owned pb contributes one
contiguous column chunk of width W<=1024:
  MM1: K=32 row-tiled split -- cols [0:512) at tile_position (0,0) and
       [512:W) at (32,0) run CONCURRENTLY (measured ~2x); features and
       their 21 contraction rows are replicated at partitions 0:32/32:64.
  exp: one op per chunk, PSUM->SBUF, assigned greedily to DVE
       (Schraudolph fp32->u16 max-trick) or ACT (exact exp via the
       activation affine) to balance the two engines -- this elementwise
       conversion is the kernel's roofline (~230 G elem/s combined).
  MM2: per segment, 42-wide stationary, the two bands' MMs issued
       adjacently as a col-tiled concurrent pair (po 0/64, measured
       232ns per 2x512 cols).
A zeroing matmul pair (42-wide zero stationary, full 512 cols, start=True)
opens every accumulator group so unwritten cells read as exact zeros --
no host-side masking.  MM2s trail one chunk (software pipeline) so exp
overlaps PE work; group evacuation [106,512]->bf16 rides the same
DVE/ACT balance.  Host does the final V*u + W*t reduction in fp64.
"""

import sys

sys.path.insert(0, "/opt/trn_rl_repo")

import numpy as np
import ml_dtypes

# ---------------- problem constants (hardcoded per contract) ---------------
N, K, H, W = 4, 21, 128, 128
OH, OW = 64, 64
P = OH * OW  # 4096
WEIGHT = 0.1
SIGMA_RGB = 15.0
SIGMA_XY = 80.0
SCALE = 0.5
IGNORE_LABEL = 255
N_CORES = 8
QB = 512
NSLOT = 16      # p-blocks per core
A_S = 128.0 / np.log(2.0)          # Schraudolph scale (bf16 bit layout)
DELTA = -7.0                       # Schraudolph bias correction (tuned)
B_S = 16256.0 + DELTA

BF16 = ml_dtypes.bfloat16

_PROGRAM = None  # built once per process


def _core_pbs(par):
    """Parity p-block split; parities have identical chunk shapes."""
    return list(range(par, 32, 2))


def _schedule(par):
    """Per band-pair group g: list of chunks (pb, W, qlo, segs); segs are
    (band, lo, hi, var) in chunk-local columns.  The diagonal chunk start
    is aligned to an EVEN sub-block so both parities share one program;
    var selects the stationary variant region: 0=S1 (diag|below-diag),
    1=S2 (above-diag|diag), 2=A (plain [t;u]).  Diagonal content is
    [t/2;u/2]: with B symmetric, V*u + W*t then counts it exactly once;
    below-diagonal content is zeros."""
    pbs = _core_pbs(par)
    groups = []
    for g in range(4):
        chunks = []
        for pb in pbs:
            if pb // 4 > 2 * g + 1:
                continue
            db = pb // 4                      # diagonal band
            c0 = 2 * ((pb % 4) // 2)          # aligned start sub-block
            segs = []
            off = 0
            qlo = None
            for b in (2 * g, 2 * g + 1):
                if b < db:
                    continue
                cb = c0 if b == db else 0
                w = 512 - cb * 128
                if qlo is None:
                    qlo = b * 512 + cb * 128
                if b == db:
                    segs.append((b, off, off + 128, 0))
                    segs.append((b, off + 128, off + 256, 1))
                    if w > 256:
                        segs.append((b, off + 256, off + w, 2))
                else:
                    segs.append((b, off, off + w, 2))
                off += w
            chunks.append((pb, off, qlo, segs))
        groups.append(chunks)
    return pbs, groups


def _hilo(x):
    x = np.asarray(x, np.float32)
    hi = x.astype(BF16)
    lo = (x - hi.astype(np.float32)).astype(BF16)
    return hi, lo


def _patch_tile_drain():
    """This container's walrus allows only one sync wait per CTRL (Drain/Nop)
    instruction; Tile's exit drain attaches one wait per DMA-HW queue sem.
    Split the extra waits onto dedicated nops."""
    from concourse import mybir
    from concourse.tile import TileContext
    from concourse.vector_clock import ScopedClock

    if getattr(TileContext, "_drain_split_patched", False):
        return

    def _drain_and_barrier(self, tick_clock, wait_clock):
        nc = self.nc
        drain_inst = nc.sync.drain()
        wait_clock.add_sem_waits(
            drain_inst.ins, ScopedClock({None: tick_clock.global_clock})
        )
        si = drain_inst.ins.sync_info
        waits = list(si.on_wait) if si is not None else []
        if len(waits) > 1:
            del si.on_wait[1:]
            for w in waits[1:]:
                n = nc.sync.nop(nofuse=True, hint="drain_split")
                n.ins.sync_info = mybir.SyncInfo(on_wait=[w], on_update=[])
        nc.all_engine_barrier()
        popped = nc._tile_sem_poison_stack.pop()
        assert popped == self._sem_poison
        nc.clear_and_free_semaphores(list(self.sems.allocated().values()))
        nc.all_engine_barrier()

    TileContext._drain_and_barrier = _drain_and_barrier
    TileContext._drain_split_patched = True


def _split_multi_waits(nc):
    """This walrus build supports one sync-wait per instruction. Hoist extra
    waits onto dedicated same-engine nops placed right before the owner."""
    from concourse import mybir

    ctr = 0
    for fn in nc.m.functions:
        for blk in fn.blocks:
            insts = blk.instructions
            new = []
            changed = False
            for inst in insts:
                si = inst.sync_info
                if si is not None and si.on_wait is not None and len(si.on_wait) > 1:
                    waits = list(si.on_wait)
                    for w in waits[:-1]:
                        ctr += 1
                        new.append(
                            mybir.InstNoOp(
                                name=f"WSPLIT-{ctr}",
                                engine=inst.engine,
                                ins=[],
                                outs=[],
                                sync_info=mybir.SyncInfo(
                                    on_wait=[w], on_update=[]
                                ),
                                text_hint="wait_split",
                                bass_nofuse=True,
                            )
                        )
                    si.on_wait = [waits[-1]]
                    inst.sync_info = si
                    changed = True
                new.append(inst)
            if changed:
                blk.instructions = new


def _build_program():
    global _PROGRAM
    if _PROGRAM is not None:
        return _PROGRAM

    _patch_tile_drain()
    import concourse.bass as bass
    from concourse import mybir
    from concourse.tile import TileContext

    nc = bass.Bass("TRN2")
    f32 = mybir.dt.float32
    bf16 = mybir.dt.bfloat16
    u16 = mybir.dt.uint16

    # flt: [64, 16*128] stationary features, 2 vertical replicas (rows
    #      0:32 / 32:64), col block s*128 = slot s's p-block, 21 real rows.
    # frt: [64, 4096] moving features, same 2 replicas.
    # st:  [128, 16*126+42]: slot s stationary variants at s*126 +
    #      {0: S1, 42: S2, 84: A}, each 42 cols = 21 t rows then 21 u rows
    #      (transposed); last 42 cols are zeros (accumulator opener).
    # out: [84, 2048]: group g at cols g*512: rows V(2g),W(2g),V(2g+1),
    #      W(2g+1) each 21 rows.
    STW = NSLOT * 126 + 42
    flt = nc.dram_tensor("flt", [64, NSLOT * 128], bf16, kind="ExternalInput")
    frt = nc.dram_tensor("frt", [64, P], bf16, kind="ExternalInput")
    st = nc.dram_tensor("st", [128, STW], bf16, kind="ExternalInput")
    out = nc.dram_tensor("out", [84, 4 * QB], bf16, kind="ExternalOutput")

    pbs, groups = _schedule(0)  # both parities share this PROGRAM shape
    slot_of = {pb: i for i, pb in enumerate(pbs)}
    pbs1, groups1 = _schedule(1)
    for g in range(4):
        assert [(c[1], [(s[1], s[2], s[3]) for s in c[3]])
                for c in groups[g]] == \
               [(c[1], [(s[1], s[2], s[3]) for s in c[3]])
                for c in groups1[g]], "core schedules diverge"

    scale = float(1.0 / A_S)
    bias = float(-B_S / A_S)

    # Register the activation bias constant (float bias needs a const AP).
    _bt = nc.alloc_sbuf_tensor("const-exp-bias", [128, 1], f32)
    nc.gpsimd.memset(_bt.ap(), bias)
    nc.const_aps.aps[(f32, bias)] = _bt.ap()
    nc.all_engine_barrier()

    # greedy DVE/ACT balance (simulated finish times, ns)
    eng_t = {"dve": 0.0, "act": 0.0}

    def exp_cost(eng, w):
        if eng == "dve":
            return w * 128 / 122.9 + 155.0
        return (w + 352) / 1.2

    with TileContext(nc) as tc:
        with (
            tc.tile_pool(name="const", bufs=1) as const,
            tc.tile_pool(name="apool", bufs=4) as apool,
            tc.tile_pool(name="osb", bufs=2) as osb,
            tc.tile_pool(name="dotps", bufs=3, space="PSUM") as dotps,
            tc.tile_pool(name="outps", bufs=1, space="PSUM") as outps,
        ):
            flt_s = const.tile([64, NSLOT * 128], bf16)
            frt_s = const.tile([64, P], bf16)
            st_s = const.tile([128, STW], bf16)

            # Input DMAs ordered by first use.
            nc.sync.dma_start(out=flt_s, in_=flt[:, :])
            nc.sync.dma_start(out=frt_s[:, 0:1024], in_=frt[:, 0:1024])
            for r in range(2):
                rs = slice(64 * r, 64 * r + 64)
                nc.sync.dma_start(out=st_s[rs, :], in_=st[rs, :])
            nc.sync.dma_start(out=frt_s[:, 1024:2048], in_=frt[:, 1024:2048])
            nc.sync.dma_start(out=frt_s[:, 2048:4096], in_=frt[:, 2048:4096])

            zcols = slice(NSLOT * 126, NSLOT * 126 + 42)

            nwr = {}        # (g, po) -> writes so far
            tot = {}        # (g, po) -> total real MM2 count
            for g in range(4):
                for pb, w, qlo, segs in groups[g]:
                    for b, lo, hi, var in segs:
                        k = (g, 64 * (b % 2))
                        tot[k] = tot.get(k, 0) + 1

            accs = {}
            pending = None

            def emit_mm2(g, acc, chunk, at):
                pb, w, qlo, segs = chunk
                s = slot_of[pb]
                # A-segs first so the two bands' MMs sit adjacent in the
                # queue (concurrent col-tiled pair); S1/S2 trail.
                segs = sorted(segs, key=lambda x: (x[3] != 2, x[3]))
                for b, lo, hi, var in segs:
                    po = 64 * (b % 2)
                    so = s * 126 + 42 * var
                    nwr[(g, po)] = nwr.get((g, po), 0) + 1
                    last = nwr[(g, po)] == tot[(g, po)]
                    bl = (qlo + lo) % 512  # band-local column offset
                    nc.tensor.matmul(
                        acc[po:po + 42, bl:bl + hi - lo],
                        lhsT=st_s[:, so:so + 42],
                        rhs=at[:, lo:hi],
                        start=False,
                        stop=last,
                        tile_position=(0, po),
                    )

            def emit_evac(g, acc):
                cs = slice(g * QB, (g + 1) * QB)
                ob = osb.tile([106, QB], bf16, tag=f"ob{g % 2}",
                              name=f"ob{g}")
                if eng_t["dve"] <= eng_t["act"]:
                    nc.vector.tensor_copy(ob, acc[0:106, :])
                    eng_t["dve"] += 690.0
                else:
                    nc.scalar.activation(
                        ob, acc[0:106, :],
                        mybir.ActivationFunctionType.Relu,
                    )
                    eng_t["act"] += 720.0
                nc.sync.dma_start(out=out[0:42, cs], in_=ob[0:42, :])
                nc.sync.dma_start(out=out[42:84, cs], in_=ob[64:106, :])

            for g in range(4):
                acc = outps.tile([128, QB], f32, tag=f"acc{g % 2}",
                                 name=f"acc{g}")
                accs[g] = acc
                for ci, chunk in enumerate(groups[g]):
                    pb, w, qlo, segs = chunk
                    s = slot_of[pb]
                    dot = dotps.tile([128, 1024], f32, tag="dot",
                                     name=f"dot{g}_{ci}")
                    w0 = min(w, 512)
                    nc.tensor.matmul(
                        dot[:, 0:w0],
                        lhsT=flt_s[0:32, s * 128:(s + 1) * 128],
                        rhs=frt_s[0:32, qlo:qlo + w0],
                        start=True, stop=True,
                        tile_position=(0, 0),
                    )
                    if w > 512:
                        nc.tensor.matmul(
                            dot[:, 512:w],
                            lhsT=flt_s[32:64, s * 128:(s + 1) * 128],
                            rhs=frt_s[32:64, qlo + 512:qlo + w],
                            start=True, stop=True,
                            tile_position=(32, 0),
                        )
                    if ci == 0:
                        # open the group's accumulator: zero stationary,
                        # full width, both po slots (concurrent pair)
                        for po in (0, 64):
                            nc.tensor.matmul(
                                acc[po:po + 42, :],
                                lhsT=st_s[:, zcols],
                                rhs=st_s[:, 0:QB],
                                start=True, stop=False,
                                tile_position=(0, po),
                            )
                    if pending is not None:
                        pg, pacc, pchunk, pat = pending
                        emit_mm2(pg, pacc, pchunk, pat)
                        if pg != g:
                            emit_evac(pg, pacc)
                    at = apool.tile([128, 1024], bf16, tag="at",
                                    name=f"at{g}_{ci}")
                    last_of_g = ci == len(groups[g]) - 1
                    cd = exp_cost("dve", w)
                    ca = exp_cost("act", w)
                    if eng_t["dve"] + cd <= eng_t["act"] + ca:
                        eng = "dve"
                    else:
                        eng = "act"
                    if last_of_g:
                        # keep the group-closing exp + evac on one engine
                        eng = "dve" if eng_t["dve"] <= eng_t["act"] else "act"
                    if eng == "dve":
                        nc.vector.tensor_scalar_max(
                            at[:, :w].bitcast(u16), dot[:, :w], 0.0
                        )
                        eng_t["dve"] += cd
                    else:
                        nc.scalar.activation(
                            at[:, :w], dot[:, :w],
                            mybir.ActivationFunctionType.Exp,
                            bias=bias, scale=scale,
                        )
                        eng_t["act"] += ca
                    pending = (g, acc, chunk, at)
            pg, pacc, pchunk, pat = pending
            emit_mm2(pg, pacc, pchunk, pat)
            emit_evac(pg, pacc)

    _split_multi_waits(nc)
    _PROGRAM = nc
    return nc


def _host_prep(images, segmentations, ROIs, seg_label):
    """Resizes, gate, t/u, scaled bilateral feature rows + hi/lo split."""
    images = np.asarray(images, np.float32)
    segmentations = np.asarray(segmentations, np.float32)
    ROIs = np.asarray(ROIs, np.float32)
    seg_label = np.asarray(seg_label, np.float32)

    # nearest resize (scale 0.5, floor(dst*2)) == [::2, ::2]
    img_s = images[:, :, ::2, ::2]
    roi_s = ROIs[:, ::2, ::2]
    lab_s = seg_label[:, 0, ::2, ::2]
    # bilinear (align_corners=False, scale 0.5) == 2x2 average pooling
    s = segmentations.reshape(N, K, OH, 2, OW, 2)
    seg_s = 0.25 * (s[:, :, :, 0, :, 0] + s[:, :, :, 0, :, 1]
                    + s[:, :, :, 1, :, 0] + s[:, :, :, 1, :, 1])

    unlabel = lab_s.astype(np.int32) == IGNORE_LABEL
    gate = roi_s - seg_s.max(axis=1)
    gate = np.where(unlabel, np.float32(1.0), gate)
    gate = np.maximum(gate, 0.0).reshape(N, P)
    seg_m = (seg_s * roi_s[:, None]).reshape(N, K, P)

    sxy = SIGMA_XY * SCALE
    ys, xs = np.meshgrid(np.arange(OH, dtype=np.float32),
                         np.arange(OW, dtype=np.float32), indexing="ij")
    xy = np.stack([xs.ravel(), ys.ravel()], axis=1) / sxy
    rgb = img_s.reshape(N, 3, P).transpose(0, 2, 1) / SIGMA_RGB
    feat = np.concatenate([np.broadcast_to(xy, (N, P, 2)), rgb],
                          axis=-1).astype(np.float32)  # [N,P,5]

    # Contraction rows: fL.T @ fR == A_S*(f.f' - .5|f|^2 - .5|f'|^2) + B_S
    alpha = np.float32(np.sqrt(A_S))
    sq = np.sum(feat * feat, axis=-1)
    af = alpha * feat
    m5 = -0.5 * sq * np.float32(A_S / 16.0)
    rows_L, rows_R = [], []
    hi, lo = _hilo(af)
    for d in range(5):  # hi*hi + hi*lo + lo*hi cross terms
        rows_L += [hi[..., d], hi[..., d], lo[..., d]]
        rows_R += [hi[..., d], lo[..., d], hi[..., d]]
    m5h, m5l = _hilo(m5)
    c16 = np.full((N, P), 16.0, BF16)
    rows_L += [m5h, m5l, c16, c16]
    rows_R += [c16, c16, m5h, m5l]
    c64 = np.full((N, P), 64.0, BF16)
    c254 = np.full((N, P), 254.0, BF16)
    dR = np.full((N, P), np.float32((B_S - 16256.0) / 64.0), BF16)
    rows_L += [c64, c64]
    rows_R += [c254, dR]
    nrows = len(rows_L)  # 21
    fLT = np.zeros((N, 32, P), BF16)
    fRT = np.zeros((N, 32, P), BF16)
    fLT[:, :nrows] = np.stack(rows_L, axis=1).astype(BF16)
    fRT[:, :nrows] = np.stack(rows_R, axis=1).astype(BF16)

    t = seg_m * gate[:, None]
    u = seg_m
    t_bf = t.astype(BF16)
    u_bf = u.astype(BF16)
    return seg_m, gate, t, u, t_bf, u_bf, fLT, fRT


def _make_in_maps(t_bf, u_bf, fLT, fRT):
    in_maps = []
    for c in range(N_CORES):
        n, par = c // 2, c % 2
        pbs = _core_pbs(par)
        flt_v = np.empty((64, NSLOT * 128), BF16)
        st_v = np.zeros((128, NSLOT * 126 + 42), BF16)
        for s, pb in enumerate(pbs):
            cols = slice(pb * 128, (pb + 1) * 128)
            blk = fLT[n][:, cols]
            flt_v[0:32, s * 128:(s + 1) * 128] = blk
            flt_v[32:64, s * 128:(s + 1) * 128] = blk
            tT = t_bf[n][:, cols].T            # [128, 21]
            uT = u_bf[n][:, cols].T
            tuh = np.concatenate([tT, uT], axis=1)          # A: [t;u]
            tu2 = np.concatenate([0.5 * tT.astype(np.float32),
                                  0.5 * uT.astype(np.float32)],
                                 axis=1).astype(BF16)       # B: [t/2;u/2]
            o = s * 126
            if par == 0:
                st_v[:, o:o + 42] = tu2        # S1 = diagonal
                st_v[:, o + 42:o + 84] = tuh   # S2 = above-diagonal
            else:
                # S1 = below-diagonal (zeros), S2 = diagonal
                st_v[:, o + 42:o + 84] = tu2
            st_v[:, o + 84:o + 126] = tuh      # A
        frt_v = np.empty((64, P), BF16)
        frt_v[0:32] = fRT[n]
        frt_v[32:64] = fRT[n]
        in_maps.append(
            {
                "flt": np.ascontiguousarray(flt_v),
                "frt": np.ascontiguousarray(frt_v),
                "st": np.ascontiguousarray(st_v),
            }
        )
    return in_maps


def _reduce_outputs(res, t, u):
    loss_tot = 0.0
    for n in range(N):
        V = np.zeros((21, P), np.float64)
        Wm = np.zeros((21, P), np.float64)
        for par in range(2):
            o = res.results[2 * n + par]["out"].astype(np.float64)
            for g in range(4):
                ch = o[:, g * QB:(g + 1) * QB]
                b0, b1 = 2 * g, 2 * g + 1
                V[:, b0 * QB:(b0 + 1) * QB] += ch[0:21]
                Wm[:, b0 * QB:(b0 + 1) * QB] += ch[21:42]
                V[:, b1 * QB:(b1 + 1) * QB] += ch[42:63]
                Wm[:, b1 * QB:(b1 + 1) * QB] += ch[63:84]
        loss_n = np.sum(V * u[n].astype(np.float64))
        loss_n += np.sum(Wm * t[n].astype(np.float64))
        loss_tot += loss_n
    loss = WEIGHT * (-0.5) * loss_tot / (N * P)
    return np.array(loss, dtype=np.float32)


def kernel(images, segmentations, ROIs, seg_label):
    from concourse.bass_utils import run_bass_kernel_spmd

    seg_m, gate, t, u, t_bf, u_bf, fLT, fRT = _host_prep(
        images, segmentations, ROIs, seg_label
    )
    nc = _build_program()
    in_maps = _make_in_maps(t_bf, u_bf, fLT, fRT)
    res = run_bass_kernel_spmd(nc, in_maps, core_ids=list(range(N_CORES)))
    return _reduce_outputs(res, t, u)
